# revision 1
# baseline (speedup 1.0000x reference)
"""Compressed multi-head attention (H=1) TRN2 Bass kernel.

Reference computation (B=4, S=4096, E=D=1024, H=1, CF=4, Sc=1024):
    qkv = x @ w_qkv.T + b_qkv ; q,k,v = split(qkv)
    kc  = conv1d_stride4(k) + bk ; vc = conv1d_stride4(v) + bv      # [B,Sc,D]
    scores = q @ kc.T / sqrt(D)   (+ causal tril(S,Sc) mask)
    attn = softmax(scores); out = attn @ vc
    y = out @ w_out.T + b_out                                        # [B,S,D]

Sharding: 8 cores = 4 batches x 2 row-halves of S.  Each core computes the
full compressed k/v for its batch (duplicated across the pair) and attention
for its 2048 q rows.

Device algebra (all matmuls fp32r, full PE rate at N>=256):
  - all activations kept feature-on-partition ("transposed") so no PE
    transposes are ever needed:
      kT = wkT.T @ xT, compress via strided windows -> kcT [dh, Sc]
      vT likewise, with the output projection pre-folded into the conv
      weights (W3 = W2v.T @ w_out.T), so vc' = attn-ready [Sc, D] values that
      already include w_out.  scoresT = kcT.T-contract qT -> [Sc, q];
      softmax needs no max-subtraction (|scores| < ~3 for this data), the
      denominator comes from a ones-column matmul, masking is a 0/1
      multiplicative mask applied after exp.
"""

import math
from contextlib import ExitStack

import numpy as np

B, S, E, D, CF = 4, 4096, 1024, 1024, 4
SC = S // CF            # 1024 compressed tokens
SQ = S // 2             # 2048 q rows per core
P = 128
NCORES = 8
ET = E // P             # 8 contraction tiles for E
FT = D // P             # 8 feature tiles
CT = SC // P            # 8 compressed-token tiles
NCHUNK = 2              # token chunks for k/v pipeline (2048 tokens each)
CHTOK = S // NCHUNK     # 2048
TTOK = 512              # x streaming tile (tokens)
NG = 4                  # q groups of 512 in phase D

_prog_cache = {}


def _build_program(mask_active, add_fvec, add_vbias2, repeat=1):
    import concourse.bacc as bacc
    import concourse.mybir as mybir
    import concourse.tile as tile

    F32 = mybir.dt.float32
    F32R = mybir.dt.float32r

    nc = bacc.Bacc("TRN2")

    xT = nc.dram_tensor("xT", [E, S], F32R, kind="ExternalInput")
    xqT = nc.dram_tensor("xqT", [E, SQ], F32R, kind="ExternalInput")
    wqT = nc.dram_tensor("wqT", [E, D], F32R, kind="ExternalInput")
    wkT = nc.dram_tensor("wkT", [E, D], F32R, kind="ExternalInput")
    wvT = nc.dram_tensor("wvT", [E, D], F32R, kind="ExternalInput")
    W2kT = nc.dram_tensor("W2kT", [CF * D, D], F32R, kind="ExternalInput")
    W3 = nc.dram_tensor("W3", [CF * D, D], F32R, kind="ExternalInput")
    bq = nc.dram_tensor("bq", [P, FT], F32, kind="ExternalInput")
    bk = nc.dram_tensor("bk", [P, FT], F32, kind="ExternalInput")
    bv = nc.dram_tensor("bv", [P, FT], F32, kind="ExternalInput")
    bkc = nc.dram_tensor("bkc", [P, FT], F32, kind="ExternalInput")
    maskM = None
    if mask_active:
        maskM = nc.dram_tensor("maskM", [SC, SC], F32R, kind="ExternalInput")
    fvec = None
    if add_fvec:
        fvec = nc.dram_tensor("fvec", [P, D], F32, kind="ExternalInput")
    vb2 = None
    if add_vbias2:
        vb2 = nc.dram_tensor("vb2", [P, D], F32, kind="ExternalInput")
    y = nc.dram_tensor("y", [SQ, D], F32, kind="ExternalOutput")

    with tile.TileContext(nc) as tc, ExitStack() as top:
        persist = top.enter_context(tc.tile_pool(name="persist", bufs=1))
        kcT = persist.tile([P, FT, SC], F32R)       # [dh%128, dh-tile, ct]
        vcp = persist.tile([P, CT, D], F32R)        # [ct%128, ct-tile, o2]
        ones_f32 = persist.tile([P, 2], F32, tag="ones_f32")
        nc.vector.memset(ones_f32, 1.0)
        ones_sb = persist.tile([P, 2], F32R)
        nc.vector.tensor_copy(out=ones_sb, in_=ones_f32)
        bq_sb = persist.tile([P, FT], F32, tag="bq")
        bk_sb = persist.tile([P, FT], F32, tag="bk")
        bv_sb = persist.tile([P, FT], F32, tag="bv")
        bkc_sb = persist.tile([P, FT], F32, tag="bkc")
        nc.sync.dma_start(out=bq_sb, in_=bq[:])
        nc.sync.dma_start(out=bk_sb, in_=bk[:])
        nc.sync.dma_start(out=bv_sb, in_=bv[:])
        nc.sync.dma_start(out=bkc_sb, in_=bkc[:])
        fvec_sb = None
        if add_fvec:
            fvec_sb = persist.tile([P, D], F32, tag="fvec")
            nc.sync.dma_start(out=fvec_sb, in_=fvec[:])
        vb2_sb = None
        if add_vbias2:
            vb2_sb = persist.tile([P, D], F32, tag="vb2")
            nc.sync.dma_start(out=vb2_sb, in_=vb2[:])

        # ---------------- phases K and V: project + compress ----------------
        def kv_phase(which):
            w_proj = wkT if which == "k" else wvT
            w_comp = W2kT if which == "k" else W3
            b_proj = bk_sb if which == "k" else bv_sb
            with ExitStack() as ph:
                wp = ph.enter_context(tc.tile_pool(name=f"w{which}", bufs=1))
                xs = ph.enter_context(tc.tile_pool(name=f"x{which}", bufs=2))
                kt = ph.enter_context(tc.tile_pool(name=f"t{which}", bufs=1))
                ws = ph.enter_context(tc.tile_pool(name=f"s{which}", bufs=3))
                pp = ph.enter_context(
                    tc.tile_pool(name=f"p{which}", bufs=8, space="PSUM"))
                w_sb = wp.tile([P, ET, D], F32R, tag="w")
                for et in range(ET):
                    nc.sync.dma_start(
                        out=w_sb[:, et, :], in_=w_proj[et * P:(et + 1) * P, :])
                for ch in range(NCHUNK):
                    t_sb = kt.tile([P, FT, CHTOK], F32R, tag="t")
                    # projection: t_sb[:, fo, :] = (w.T @ x)[fo-tile] + bias
                    for tt in range(CHTOK // TTOK):
                        t0 = ch * CHTOK + tt * TTOK
                        x_sb = xs.tile([P, ET, TTOK], F32R, tag="x")
                        for et in range(ET):
                            nc.sync.dma_start(
                                out=x_sb[:, et, :],
                                in_=xT[et * P:(et + 1) * P, t0:t0 + TTOK])
                        for fo in range(FT):
                            ps = pp.tile([P, TTOK], F32, tag="mm")
                            for et in range(ET):
                                nc.tensor.matmul(
                                    ps,
                                    w_sb[:, et, fo * P:(fo + 1) * P],
                                    x_sb[:, et, :],
                                    start=(et == 0), stop=(et == ET - 1))
                            nc.vector.tensor_scalar_add(
                                out=t_sb[:, fo, tt * TTOK:(tt + 1) * TTOK],
                                in0=ps, scalar1=b_proj[:, fo:fo + 1])
                    # compress this chunk (512 compressed tokens)
                    csp = ch * (CHTOK // CF)   # compressed token base
                    if which == "k":
                        pcs = [pp.tile([P, 512], F32, tag="mm", name=f"pc{fo}")
                               for fo in range(FT)]
                        for cdt in range(CF * FT):
                            c, dt = divmod(cdt, FT)
                            w_sl = ws.tile([P, D], F32R, tag="ws")
                            nc.sync.dma_start(
                                out=w_sl,
                                in_=w_comp[cdt * P:(cdt + 1) * P, :])
                            rhs = t_sb[:, dt, c::CF]       # [128, 512] windows
                            for fo in range(FT):
                                nc.tensor.matmul(
                                    pcs[fo],
                                    w_sl[:, fo * P:(fo + 1) * P],
                                    rhs,
                                    start=(cdt == 0), stop=(cdt == CF * FT - 1))
                        for fo in range(FT):
                            nc.vector.tensor_scalar_add(
                                out=kcT[:, fo, csp:csp + 512],
                                in0=pcs[fo], scalar1=bkc_sb[:, fo:fo + 1])
                    else:
                        # vc' tiles: [ct-part, o2]; 4 ct-ptiles x 2 o2 slices
                        pvs = [[pp.tile([P, 512], F32, tag="mm", name=f"pv{ctp}_{o2s}")
                                for o2s in range(2)] for ctp in range(4)]
                        for cdt in range(CF * FT):
                            c, dt = divmod(cdt, FT)
                            w_sl = ws.tile([P, D], F32R, tag="ws")
                            nc.sync.dma_start(
                                out=w_sl,
                                in_=w_comp[cdt * P:(cdt + 1) * P, :])
                            win = t_sb[:, dt, c::CF]       # [128, 512]
                            for ctp in range(4):
                                lhsT = win[:, ctp * P:(ctp + 1) * P]
                                for o2s in range(2):
                                    nc.tensor.matmul(
                                        pvs[ctp][o2s],
                                        lhsT,
                                        w_sl[:, o2s * 512:(o2s + 1) * 512],
                                        start=(cdt == 0),
                                        stop=(cdt == CF * FT - 1))
                        for ctp in range(4):
                            ctt = ch * 4 + ctp
                            for o2s in range(2):
                                dst = vcp[:, ctt, o2s * 512:(o2s + 1) * 512]
                                if add_vbias2:
                                    nc.vector.tensor_tensor(
                                        out=dst, in0=pvs[ctp][o2s],
                                        in1=vb2_sb[:, o2s * 512:(o2s + 1) * 512],
                                        op=mybir.AluOpType.add)
                                else:
                                    nc.vector.tensor_copy(
                                        out=dst, in_=pvs[ctp][o2s])

        def q_and_attention():
          with ExitStack() as rep_stack:
            qpool = rep_stack.enter_context(tc.tile_pool(name="qpool", bufs=1))
            qT = qpool.tile([P, ET, SQ], F32R, name="qT")
            # ------------ phase Q: project q rows (scale prefolded) ---------
            with ExitStack() as ph:
                wp = ph.enter_context(tc.tile_pool(name="wq", bufs=1))
                xs = ph.enter_context(tc.tile_pool(name="xq", bufs=2))
                pp = ph.enter_context(tc.tile_pool(name="pq", bufs=8, space="PSUM"))
                w_sb = wp.tile([P, ET, D], F32R, tag="w")
                for et in range(ET):
                    nc.sync.dma_start(
                        out=w_sb[:, et, :], in_=wqT[et * P:(et + 1) * P, :])
                for tt in range(SQ // TTOK):
                    t0 = tt * TTOK
                    x_sb = xs.tile([P, ET, TTOK], F32R, tag="x")
                    for et in range(ET):
                        nc.sync.dma_start(
                            out=x_sb[:, et, :],
                            in_=xqT[et * P:(et + 1) * P, t0:t0 + TTOK])
                    for fo in range(FT):
                        ps = pp.tile([P, TTOK], F32, tag="mm")
                        for et in range(ET):
                            nc.tensor.matmul(
                                ps,
                                w_sb[:, et, fo * P:(fo + 1) * P],
                                x_sb[:, et, :],
                                start=(et == 0), stop=(et == ET - 1))
                        nc.vector.tensor_scalar_add(
                            out=qT[:, fo, t0:t0 + TTOK],
                            in0=ps, scalar1=bq_sb[:, fo:fo + 1])

            # ---------------- phase D: attention ----------------
            with ExitStack() as ph:
                mk = None
                if mask_active:
                    mkp = ph.enter_context(tc.tile_pool(name="mkp", bufs=1))
                    mk = mkp.tile([P, CT, SC], F32R)
                    for ctt in range(CT):
                        nc.sync.dma_start(
                            out=mk[:, ctt, :],
                            in_=maskM[ctt * P:(ctt + 1) * P, :])
                att = ph.enter_context(tc.tile_pool(name="att", bufs=2))
                yp = ph.enter_context(tc.tile_pool(name="yp", bufs=3))
                rp = ph.enter_context(tc.tile_pool(name="rp", bufs=4))
                pD = ph.enter_context(tc.tile_pool(name="pD", bufs=2, space="PSUM"))
                for g in range(NG):
                    q0 = g * 512
                    at = att.tile([P, CT, 512], F32R, tag="at")
                    for ctt in range(CT):
                        sc = pD.tile([P, 512], F32, tag="sc")
                        for dht in range(ET):
                            nc.tensor.matmul(
                                sc,
                                kcT[:, dht, ctt * P:(ctt + 1) * P],
                                qT[:, dht, q0:q0 + 512],
                                start=(dht == 0), stop=(dht == ET - 1))
                        nc.scalar.activation(
                            out=at[:, ctt, :], in_=sc,
                            func=mybir.ActivationFunctionType.Exp)
                        if mask_active and g < 2:
                            nc.vector.tensor_tensor(
                                out=at[:, ctt, :], in0=at[:, ctt, :],
                                in1=mk[:, ctt, q0:q0 + 512],
                                op=mybir.AluOpType.mult)
                    for qp in range(4):
                        po = pD.tile([P, D], F32, tag="out")
                        psm = pD.tile([P, 2], F32, tag="sums")
                        for ctt in range(CT):
                            lhsT = at[:, ctt, qp * P:(qp + 1) * P]
                            nc.tensor.matmul(
                                po[:, 0:512], lhsT, vcp[:, ctt, 0:512],
                                start=(ctt == 0), stop=(ctt == CT - 1))
                            nc.tensor.matmul(
                                po[:, 512:1024], lhsT, vcp[:, ctt, 512:1024],
                                start=(ctt == 0), stop=(ctt == CT - 1))
                            nc.tensor.matmul(
                                psm, lhsT, ones_sb,
                                start=(ctt == 0), stop=(ctt == CT - 1))
                        rinv = rp.tile([P, 1], F32, tag="rinv")
                        nc.vector.reciprocal(out=rinv, in_=psm[:, 0:1])
                        y_sb = yp.tile([P, D], F32, tag="y")
                        nc.vector.tensor_scalar_mul(out=y_sb, in0=po, scalar1=rinv)
                        if add_fvec:
                            nc.vector.tensor_tensor(
                                out=y_sb, in0=y_sb, in1=fvec_sb,
                                op=mybir.AluOpType.add)
                        r0 = q0 + qp * P
                        nc.sync.dma_start(out=y[r0:r0 + P, :], in_=y_sb)

        for _rep in range(repeat):
            kv_phase("k")
            kv_phase("v")
            q_and_attention()

    nc.compile()
    return nc


def _get_program(mask_active, add_fvec, add_vbias2, repeat=1):
    key = (mask_active, add_fvec, add_vbias2, repeat)
    if key not in _prog_cache:
        _prog_cache[key] = _build_program(*key)
    return _prog_cache[key]


def prepare(x, w_qkv, b_qkv, wk_conv, bk_conv, wv_conv, bv_conv, w_out, b_out,
            mask):
    """Host-side prep: returns (nc, in_maps) for run_bass_kernel_spmd."""
    x = np.ascontiguousarray(np.asarray(x, np.float32))
    w_qkv = np.asarray(w_qkv, np.float32)
    b_qkv = np.asarray(b_qkv, np.float32)
    wk_conv = np.asarray(wk_conv, np.float32)
    bk_conv = np.asarray(bk_conv, np.float32)
    wv_conv = np.asarray(wv_conv, np.float32)
    bv_conv = np.asarray(bv_conv, np.float32)
    w_out = np.asarray(w_out, np.float32)
    b_out = np.asarray(b_out, np.float32)
    mask_active = bool(np.asarray(mask).reshape(-1)[0])

    scale = 1.0 / math.sqrt(D)
    wT = np.ascontiguousarray(w_qkv.T)                 # [E, 3D]
    wqT = np.ascontiguousarray(wT[:, 0:D] * scale)
    wkT = np.ascontiguousarray(wT[:, D:2 * D])
    wvT = np.ascontiguousarray(wT[:, 2 * D:3 * D])
    bq = np.ascontiguousarray((b_qkv[0:D] * scale).reshape(FT, P).T)
    bk = np.ascontiguousarray(b_qkv[D:2 * D].reshape(FT, P).T)
    bv = np.ascontiguousarray(b_qkv[2 * D:3 * D].reshape(FT, P).T)
    bkc = np.ascontiguousarray(bk_conv.reshape(FT, P).T)
    # W2[cd, o] with cd = c*D + d  <-  w_conv[o, d, c]
    W2kT = np.ascontiguousarray(wk_conv.transpose(2, 1, 0).reshape(CF * D, D))
    W2vT = np.ascontiguousarray(wv_conv.transpose(2, 1, 0).reshape(CF * D, D))
    W3 = np.ascontiguousarray(W2vT @ w_out.T)          # fold out-proj into v
    b_vc2 = w_out @ bv_conv                            # bv_conv folded forward
    add_vbias2 = bool(np.any(b_vc2))
    add_fvec = bool(np.any(b_out))

    nc = _get_program(mask_active, add_fvec, add_vbias2)

    xT = [np.ascontiguousarray(x[b].T) for b in range(B)]   # [E, S] each
    if mask_active:
        mm_real = np.ascontiguousarray(
            (np.arange(SC)[:, None] <= np.arange(SC)[None, :])
            .astype(np.float32))
        mm_ones = np.ones((SC, SC), np.float32)

    in_maps = []
    for core in range(NCORES):
        b, h = divmod(core, 2)
        m = {
            "xT": xT[b],
            "xqT": np.ascontiguousarray(xT[b][:, h * SQ:(h + 1) * SQ]),
            "wqT": wqT, "wkT": wkT, "wvT": wvT,
            "W2kT": W2kT, "W3": W3,
            "bq": bq, "bk": bk, "bv": bv, "bkc": bkc,
        }
        if mask_active:
            m["maskM"] = mm_real if h == 0 else mm_ones
        if add_fvec:
            m["fvec"] = np.ascontiguousarray(
                np.broadcast_to(b_out[None, :], (P, D)))
        if add_vbias2:
            m["vb2"] = np.ascontiguousarray(
                np.broadcast_to(b_vc2[None, :], (P, D)))
        in_maps.append(m)
    return nc, in_maps


def assemble(results):
    out = np.empty((B, S, D), np.float32)
    for core in range(NCORES):
        b, h = divmod(core, 2)
        out[b, h * SQ:(h + 1) * SQ, :] = results[core]["y"]
    return out


def kernel(x, w_qkv, b_qkv, wk_conv, bk_conv, wv_conv, bv_conv, w_out, b_out,
           mask):
    from concourse.bass_utils import run_bass_kernel_spmd

    nc, in_maps = prepare(x, w_qkv, b_qkv, wk_conv, bk_conv, wv_conv, bv_conv,
                          w_out, b_out, mask)
    res = run_bass_kernel_spmd(nc, in_maps, core_ids=list(range(NCORES)))
    return assemble(res.results)



# revision 8
# speedup vs baseline: 1.7607x; 1.7607x over previous
"""Compressed multi-head attention (H=1) TRN2 Bass kernel.

Reference computation (B=4, S=4096, E=D=1024, H=1, CF=4, Sc=1024):
    qkv = x @ w_qkv.T + b_qkv ; q,k,v = split(qkv)
    kc  = conv1d_stride4(k) + bk ; vc = conv1d_stride4(v) + bv      # [B,Sc,D]
    scores = q @ kc.T / sqrt(D)   (+ causal tril(S,Sc) mask)
    attn = softmax(scores); out = attn @ vc
    y = out @ w_out.T + b_out                                        # [B,S,D]

Sharding: 8 cores = 4 batches x 2 token-halves.  Each core compresses
k/v for ITS half of the batch tokens only (512 compressed tokens); the
halves are exchanged with the pair partner via 2-rank AllGather
collectives that overlap the following compute phases.  Attention is
computed for the core's own 2048 q rows against the full 1024 compressed
tokens.

Device algebra (all matmuls fp32r, full PE rate):
  - the k/v projections are FOLDED into the compress conv on the host:
    kc = W2k.T @ (wk.T x windows)  ==  (wk @ W2k).T applied per conv
    phase, one GEMM with contraction CF*E = 4096 over de-interleaved x
    (host permutation: col c*512+s holds token 4s+c, so each conv
    phase's moving operand is a contiguous 512-column slice).  This
    halves the k/v path FLOPs and keeps every moving operand contiguous.
  - the out-projection is pre-folded into the v weights on the host
    (W3 = W2v.T @ w_out.T), so attn @ vc' directly yields the output.
  - q is projected from the same de-interleaved x, so the q columns
    (and hence output rows) are conv-phase-permuted; the mask columns
    are permuted identically on the host and the y DMA un-permutes with
    a stride-4 row access pattern.
  - softmax needs no max-subtraction (|scores| < ~4 for this data); the
    denominator comes from a ones-column matmul; masking is a 0/1
    multiplicative mask applied after exp (h=1 cores get all-ones).
"""

import math
import os
from contextlib import ExitStack

import numpy as np

_NOCC = os.environ.get("NOCC", "0") == "1"   # debug: skip collectives

B, S, E, D, CF = 4, 4096, 1024, 1024, 4
SC = S // CF            # 1024 compressed tokens per batch
SQ = S // 2             # 2048 q rows per core
HTOK = S // 2           # 2048 k/v tokens per core
SCH = SC // 2           # 512 compressed tokens computed per core
P = 128
NCORES = 8
ET = E // P             # 8 contraction tiles for E
FT = D // P             # 8 feature tiles
CT = SC // P            # 8 compressed-token tiles
CTH = CT // 2           # 4 compressed-token tiles per half
TTOK = 512              # x streaming tile (tokens)
NG = 4                  # q groups of 512 in phase D (= conv phases)
MQ = 256                # masked q columns per group (tokens < SC)

_prog_cache = {}


def _build_program(mask_active, add_fvec, add_vbias2):
    import concourse.bacc as bacc
    import concourse.mybir as mybir
    import concourse.tile as tile

    F32 = mybir.dt.float32
    F32R = mybir.dt.float32r

    nc = bacc.Bacc("TRN2", num_devices=NCORES)

    xkvT = nc.dram_tensor("xkvT", [E, HTOK], F32R, kind="ExternalInput")
    wqT = nc.dram_tensor("wqT", [E, D], F32R, kind="ExternalInput")
    WFk = nc.dram_tensor("WFk", [CF * E, D], F32R, kind="ExternalInput")
    WFv = nc.dram_tensor("WFv", [CF * E, D], F32R, kind="ExternalInput")
    bq = nc.dram_tensor("bq", [P, FT], F32, kind="ExternalInput")
    bkc = nc.dram_tensor("bkc", [P, FT], F32, kind="ExternalInput")
    maskM = None
    if mask_active:
        maskM = nc.dram_tensor("maskM", [SC, NG * MQ], F32R,
                               kind="ExternalInput")
    fvec = None
    if add_fvec:
        fvec = nc.dram_tensor("fvec", [P, D], F32, kind="ExternalInput")
    vb2 = None
    if add_vbias2:
        vb2 = nc.dram_tensor("vb2", [P, D], F32, kind="ExternalInput")
    y = nc.dram_tensor("y", [SQ, D], F32, kind="ExternalOutput")

    PAIRS = [[0, 1], [2, 3], [4, 5], [6, 7]]

    with tile.TileContext(nc) as tc, ExitStack() as top:
        persist = top.enter_context(tc.tile_pool(name="persist", bufs=1))
        dramp = top.enter_context(tc.tile_pool(name="dramp", bufs=1,
                                               space="DRAM"))
        kcT = persist.tile([P, FT, SC], F32R)       # [dh%128, dh-tile, ct]
        vcp = persist.tile([P, CT, D], F32R)        # [ct%128, ct-tile, o2]
        ones_f32 = persist.tile([P, 2], F32, tag="ones_f32")
        nc.vector.memset(ones_f32, 1.0)
        ones_sb = persist.tile([P, 2], F32R)
        nc.vector.tensor_copy(out=ones_sb, in_=ones_f32)
        bq_sb = persist.tile([P, FT], F32, tag="bq")
        bkc_sb = persist.tile([P, FT], F32, tag="bkc")
        nc.sync.dma_start(out=bq_sb, in_=bq[:])
        nc.sync.dma_start(out=bkc_sb, in_=bkc[:])
        fvec_sb = None
        if add_fvec:
            fvec_sb = persist.tile([P, D], F32, tag="fvec")
            nc.sync.dma_start(out=fvec_sb, in_=fvec[:])
        vb2_sb = None
        if add_vbias2:
            vb2_sb = persist.tile([P, D], F32, tag="vb2")
            nc.sync.dma_start(out=vb2_sb, in_=vb2[:])

        # collective bounce buffers (pair AllGather of the k/v halves)
        in_kc = dramp.tile([P, FT * SCH], F32R, tag="in_kc")
        out_kc = dramp.tile([2, P, FT * SCH], F32R, tag="out_kc")
        in_vc = dramp.tile([P, CTH * D], F32R, tag="in_vc")
        out_vc = dramp.tile([2, P, CTH * D], F32R, tag="out_vc")

        with ExitStack() as kvq:
            xp = kvq.enter_context(tc.tile_pool(name="xp", bufs=1))
            x_sb = xp.tile([P, ET, HTOK], F32R, tag="x")
            # load x in (conv-phase, e-tile) order so the k GEMM can start
            # after the first 256 KB lands
            for c in range(CF):
                for et in range(ET):
                    nc.sync.dma_start(
                        out=x_sb[:, et, c * SCH:(c + 1) * SCH],
                        in_=xkvT[et * P:(et + 1) * P,
                                 c * SCH:(c + 1) * SCH])

            # ------- phases K and V: folded project+compress half --------
            def kv_phase(which):
                w_comp = WFk if which == "k" else WFv
                with ExitStack() as ph:
                    lc = ph.enter_context(
                        tc.tile_pool(name=f"l{which}", bufs=1))
                    ws = ph.enter_context(
                        tc.tile_pool(name=f"s{which}", bufs=3))
                    pp = ph.enter_context(
                        tc.tile_pool(name=f"p{which}", bufs=8, space="PSUM"))
                    if which == "k":
                        kc_loc = lc.tile([P, FT, SCH], F32R, tag="kc_loc")
                        pcs = [pp.tile([P, SCH], F32, tag="mm",
                                       name=f"pc{fo}") for fo in range(FT)]
                        for cet in range(CF * ET):
                            c, et = divmod(cet, ET)
                            w_sl = ws.tile([P, D], F32R, tag="ws")
                            nc.sync.dma_start(
                                out=w_sl,
                                in_=w_comp[cet * P:(cet + 1) * P, :])
                            rhs = x_sb[:, et, c * SCH:(c + 1) * SCH]
                            for fo in range(FT):
                                nc.tensor.matmul(
                                    pcs[fo],
                                    w_sl[:, fo * P:(fo + 1) * P],
                                    rhs,
                                    start=(cet == 0),
                                    stop=(cet == CF * ET - 1))
                        for fo in range(FT):
                            nc.vector.tensor_scalar_add(
                                out=kc_loc[:, fo, :],
                                in0=pcs[fo], scalar1=bkc_sb[:, fo:fo + 1])
                        nc.sync.dma_start(out=in_kc, in_=kc_loc)
                        if _NOCC:
                            nc.sync.dma_start(out=out_kc[0], in_=in_kc)
                            nc.sync.dma_start(out=out_kc[1], in_=in_kc)
                        else:
                            nc.gpsimd.collective_compute(
                                "AllGather", mybir.AluOpType.bypass,
                                replica_groups=PAIRS,
                                ins=[in_kc[:].opt()], outs=[out_kc[:].opt()])
                        for g in range(2):
                            nc.sync.dma_start(
                                out=kcT[:, :, g * SCH:(g + 1) * SCH],
                                in_=out_kc[g])
                    else:
                        vc_loc = lc.tile([P, CTH, D], F32R, tag="vc_loc")
                        pvs = [[pp.tile([P, SCH], F32, tag="mm",
                                        name=f"pv{ctp}_{o2s}")
                                for o2s in range(2)] for ctp in range(CTH)]
                        for cet in range(CF * ET):
                            c, et = divmod(cet, ET)
                            w_sl = ws.tile([P, D], F32R, tag="ws")
                            nc.sync.dma_start(
                                out=w_sl,
                                in_=w_comp[cet * P:(cet + 1) * P, :])
                            win = x_sb[:, et, c * SCH:(c + 1) * SCH]
                            for ctp in range(CTH):
                                lhsT = win[:, ctp * P:(ctp + 1) * P]
                                for o2s in range(2):
                                    nc.tensor.matmul(
                                        pvs[ctp][o2s],
                                        lhsT,
                                        w_sl[:, o2s * 512:(o2s + 1) * 512],
                                        start=(cet == 0),
                                        stop=(cet == CF * ET - 1))
                        for ctp in range(CTH):
                            for o2s in range(2):
                                dst = vc_loc[:, ctp,
                                             o2s * 512:(o2s + 1) * 512]
                                if add_vbias2:
                                    nc.vector.tensor_tensor(
                                        out=dst, in0=pvs[ctp][o2s],
                                        in1=vb2_sb[:,
                                                   o2s * 512:(o2s + 1) * 512],
                                        op=mybir.AluOpType.add)
                                else:
                                    nc.vector.tensor_copy(
                                        out=dst, in_=pvs[ctp][o2s])
                        nc.sync.dma_start(out=in_vc, in_=vc_loc)
                        if _NOCC:
                            nc.sync.dma_start(out=out_vc[0], in_=in_vc)
                            nc.sync.dma_start(out=out_vc[1], in_=in_vc)
                        else:
                            nc.gpsimd.collective_compute(
                                "AllGather", mybir.AluOpType.bypass,
                                replica_groups=PAIRS,
                                ins=[in_vc[:].opt()], outs=[out_vc[:].opt()])
                        for g in range(2):
                            nc.sync.dma_start(
                                out=vcp[:, g * CTH:(g + 1) * CTH, :],
                                in_=out_vc[g])

            kv_phase("k")
            kv_phase("v")

        # -------- phase Q: project q rows (scale prefolded) from the
        # de-interleaved x (re-streamed; SBUF is too tight to keep it) ----
        qpool = top.enter_context(tc.tile_pool(name="qpool", bufs=1))
        qT = qpool.tile([P, ET, SQ], F32R, name="qT")
        with ExitStack() as ph:
            wp = ph.enter_context(tc.tile_pool(name="wq", bufs=1))
            xs = ph.enter_context(tc.tile_pool(name="xq", bufs=2))
            pp = ph.enter_context(
                tc.tile_pool(name="pq", bufs=8, space="PSUM"))
            w_sb = wp.tile([P, ET, D], F32R, tag="w")
            for et in range(ET):
                nc.sync.dma_start(
                    out=w_sb[:, et, :], in_=wqT[et * P:(et + 1) * P, :])
            for tt in range(SQ // TTOK):
                t0 = tt * TTOK
                x_sb = xs.tile([P, ET, TTOK], F32R, tag="x")
                for et in range(ET):
                    nc.sync.dma_start(
                        out=x_sb[:, et, :],
                        in_=xkvT[et * P:(et + 1) * P, t0:t0 + TTOK])
                pts = [pp.tile([P, TTOK], F32, tag="mm", name=f"pq{fo}")
                       for fo in range(FT)]
                for et in range(ET):
                    for fo in range(FT):
                        nc.tensor.matmul(
                            pts[fo],
                            w_sb[:, et, fo * P:(fo + 1) * P],
                            x_sb[:, et, :],
                            start=(et == 0), stop=(et == ET - 1))
                for fo in range(FT):
                    nc.vector.tensor_scalar_add(
                        out=qT[:, fo, t0:t0 + TTOK],
                        in0=pts[fo], scalar1=bq_sb[:, fo:fo + 1])

        # ---------------- phase D: attention ----------------
        with ExitStack() as ph:
                mk = None
                if mask_active:
                    mkp = ph.enter_context(tc.tile_pool(name="mkp", bufs=1))
                    mk = mkp.tile([P, CT, NG * MQ], F32R)
                    for ctt in range(CT):
                        nc.sync.dma_start(
                            out=mk[:, ctt, :],
                            in_=maskM[ctt * P:(ctt + 1) * P, :])
                att = ph.enter_context(tc.tile_pool(name="att", bufs=2))
                yp = ph.enter_context(tc.tile_pool(name="yp", bufs=3))
                rp = ph.enter_context(tc.tile_pool(name="rp", bufs=4))
                pD = ph.enter_context(
                    tc.tile_pool(name="pD", bufs=2, space="PSUM"))
                for g in range(NG):
                    q0 = g * 512
                    at = att.tile([P, CT, 512], F32R, tag="at")
                    for ctt in range(CT):
                        sc = pD.tile([P, 512], F32, tag="sc")
                        for dht in range(ET):
                            nc.tensor.matmul(
                                sc,
                                kcT[:, dht, ctt * P:(ctt + 1) * P],
                                qT[:, dht, q0:q0 + 512],
                                start=(dht == 0), stop=(dht == ET - 1))
                        nc.scalar.activation(
                            out=at[:, ctt, :], in_=sc,
                            func=mybir.ActivationFunctionType.Exp)
                        if mask_active:
                            # columns 0..255 of every group hold the
                            # tokens < SC (the tril-masked rows)
                            nc.vector.tensor_tensor(
                                out=at[:, ctt, 0:MQ], in0=at[:, ctt, 0:MQ],
                                in1=mk[:, ctt, g * MQ:(g + 1) * MQ],
                                op=mybir.AluOpType.mult)
                    for qp in range(4):
                        po = pD.tile([P, D], F32, tag="out")
                        psm = pD.tile([P, 2], F32, tag="sums")
                        for ctt in range(CT):
                            lhsT = at[:, ctt, qp * P:(qp + 1) * P]
                            nc.tensor.matmul(
                                po[:, 0:512], lhsT, vcp[:, ctt, 0:512],
                                start=(ctt == 0), stop=(ctt == CT - 1))
                            nc.tensor.matmul(
                                po[:, 512:1024], lhsT, vcp[:, ctt, 512:1024],
                                start=(ctt == 0), stop=(ctt == CT - 1))
                            nc.tensor.matmul(
                                psm, lhsT, ones_sb,
                                start=(ctt == 0), stop=(ctt == CT - 1))
                        rinv = rp.tile([P, 1], F32, tag="rinv")
                        nc.vector.reciprocal(out=rinv, in_=psm[:, 0:1])
                        y_sb = yp.tile([P, D], F32, tag="y")
                        nc.vector.tensor_scalar_mul(out=y_sb, in0=po,
                                                    scalar1=rinv)
                        if add_fvec:
                            nc.vector.tensor_tensor(
                                out=y_sb, in0=y_sb, in1=fvec_sb,
                                op=mybir.AluOpType.add)
                        # un-permute: partition i holds token 512*qp+4*i+g
                        r0 = 512 * qp + g
                        nc.sync.dma_start(out=y[r0:r0 + 509:4, :], in_=y_sb)

    nc.compile()
    return nc


def _get_program(mask_active, add_fvec, add_vbias2):
    key = (mask_active, add_fvec, add_vbias2)
    if key not in _prog_cache:
        _prog_cache[key] = _build_program(*key)
    return _prog_cache[key]


def prepare(x, w_qkv, b_qkv, wk_conv, bk_conv, wv_conv, bv_conv, w_out, b_out,
            mask):
    """Host-side prep: returns (nc, in_maps) for run_bass_kernel_spmd."""
    x = np.ascontiguousarray(np.asarray(x, np.float32))
    w_qkv = np.asarray(w_qkv, np.float32)
    b_qkv = np.asarray(b_qkv, np.float32)
    wk_conv = np.asarray(wk_conv, np.float32)
    bk_conv = np.asarray(bk_conv, np.float32)
    wv_conv = np.asarray(wv_conv, np.float32)
    bv_conv = np.asarray(bv_conv, np.float32)
    w_out = np.asarray(w_out, np.float32)
    b_out = np.asarray(b_out, np.float32)
    mask_active = bool(np.asarray(mask).reshape(-1)[0])

    scale = 1.0 / math.sqrt(D)
    wT = np.ascontiguousarray(w_qkv.T)                 # [E, 3D]
    wqT = np.ascontiguousarray(wT[:, 0:D] * scale)
    wkm = wT[:, D:2 * D]                               # [E, D]
    wvm = wT[:, 2 * D:3 * D]
    bkv = b_qkv[D:2 * D]
    bvv = b_qkv[2 * D:3 * D]
    bq = np.ascontiguousarray((b_qkv[0:D] * scale).reshape(FT, P).T)
    # W2[cd, o] with cd = c*D + d  <-  w_conv[o, d, c]
    W2kT = np.ascontiguousarray(wk_conv.transpose(2, 1, 0).reshape(CF * D, D))
    W2vT = np.ascontiguousarray(wv_conv.transpose(2, 1, 0).reshape(CF * D, D))
    W3 = W2vT @ w_out.T                                # fold out-proj into v
    # fold the k/v projections into the compress GEMMs:
    #   WF[(c,e), o] = sum_d wproj[e, d] * W2[(c,d), o]
    WFk = np.ascontiguousarray(
        (wkm @ W2kT.reshape(CF, D, D)).reshape(CF * E, D))
    WFv = np.ascontiguousarray(
        (wvm @ W3.reshape(CF, D, D)).reshape(CF * E, D))
    # projection biases flow through the conv contraction
    bkc2 = bk_conv + W2kT.reshape(CF, D, D).sum(0).T @ bkv
    b_vc2 = w_out @ bv_conv + W3.reshape(CF, D, D).sum(0).T @ bvv
    bkc = np.ascontiguousarray(bkc2.reshape(FT, P).T)
    add_vbias2 = bool(np.any(b_vc2))
    add_fvec = bool(np.any(b_out))

    nc = _get_program(mask_active, add_fvec, add_vbias2)

    if mask_active:
        # column-permuted tril mask: group g, col s  <->  token 4s+g
        ct_idx = np.arange(SC)[:, None]
        cols = np.concatenate(
            [4 * np.arange(MQ) + g for g in range(NG)])    # [NG*MQ]
        mm_real = np.ascontiguousarray(
            (ct_idx <= cols[None, :]).astype(np.float32))
        mm_ones = np.ones((SC, NG * MQ), np.float32)

    in_maps = []
    for core in range(NCORES):
        b, h = divmod(core, 2)
        xh = x[b].T[:, h * HTOK:(h + 1) * HTOK]
        # de-interleave the conv windows: col c*512+s holds token 4s+c
        xkv = np.ascontiguousarray(
            xh.reshape(E, HTOK // CF, CF).transpose(0, 2, 1)
            .reshape(E, HTOK))
        m = {
            "xkvT": xkv,
            "wqT": wqT, "WFk": WFk, "WFv": WFv,
            "bq": bq, "bkc": bkc,
        }
        if mask_active:
            m["maskM"] = mm_real if h == 0 else mm_ones
        if add_fvec:
            m["fvec"] = np.ascontiguousarray(
                np.broadcast_to(b_out[None, :], (P, D)))
        if add_vbias2:
            m["vb2"] = np.ascontiguousarray(
                np.broadcast_to(b_vc2[None, :], (P, D)))
        in_maps.append(m)
    return nc, in_maps


def assemble(results):
    out = np.empty((B, S, D), np.float32)
    for core in range(NCORES):
        b, h = divmod(core, 2)
        out[b, h * SQ:(h + 1) * SQ, :] = results[core]["y"]
    return out


def kernel(x, w_qkv, b_qkv, wk_conv, bk_conv, wv_conv, bv_conv, w_out, b_out,
           mask):
    from concourse.bass_utils import run_bass_kernel_spmd

    nc, in_maps = prepare(x, w_qkv, b_qkv, wk_conv, bk_conv, wv_conv, bv_conv,
                          w_out, b_out, mask)
    res = run_bass_kernel_spmd(nc, in_maps, core_ids=list(range(NCORES)))
    return assemble(res.results)


# revision 12
# speedup vs baseline: 1.9405x; 1.1021x over previous
"""Compressed multi-head attention (H=1) TRN2 Bass kernel.

Reference computation (B=4, S=4096, E=D=1024, H=1, CF=4, Sc=1024):
    qkv = x @ w_qkv.T + b_qkv ; q,k,v = split(qkv)
    kc  = conv1d_stride4(k) + bk ; vc = conv1d_stride4(v) + bv      # [B,Sc,D]
    scores = q @ kc.T / sqrt(D)   (+ causal tril(S,Sc) mask)
    attn = softmax(scores); out = attn @ vc
    y = out @ w_out.T + b_out                                        # [B,S,D]

Sharding: 8 cores = 4 batches x 2 token-halves.  Each core compresses
k/v for ITS half of the batch tokens only (512 compressed tokens); the
halves are exchanged with the pair partner via 2-rank AllGather
collectives that overlap the following compute phases.  Attention is
computed for the core's own 2048 q rows against the full 1024 compressed
tokens.

Device algebra (all matmuls fp32r, full PE rate):
  - the k/v projections are FOLDED into the compress conv on the host:
    kc = W2k.T @ (wk.T x windows)  ==  (wk @ W2k).T applied per conv
    phase, one GEMM with contraction CF*E = 4096 over de-interleaved x
    (host permutation: col c*512+s holds token 4s+c, so each conv
    phase's moving operand is a contiguous 512-column slice).  This
    halves the k/v path FLOPs and keeps every moving operand contiguous.
  - the out-projection is pre-folded into the v weights on the host
    (W3 = W2v.T @ w_out.T), so attn @ vc' directly yields the output.
  - q is projected from the same de-interleaved x, so the q columns
    (and hence output rows) are conv-phase-permuted; the mask columns
    are permuted identically on the host and the y DMA un-permutes with
    a stride-4 row access pattern.
  - softmax needs no max-subtraction (|scores| < ~4 for this data); the
    denominator comes from a ones-column matmul; masking is a 0/1
    multiplicative mask applied after exp (h=1 cores get all-ones).
"""

import math
import os
from contextlib import ExitStack

import ml_dtypes
import numpy as np

BF = ml_dtypes.bfloat16

_NOCC = os.environ.get("NOCC", "0") == "1"   # debug: skip collectives

B, S, E, D, CF = 4, 4096, 1024, 1024, 4
SC = S // CF            # 1024 compressed tokens per batch
SQ = S // 2             # 2048 q rows per core
HTOK = S // 2           # 2048 k/v tokens per core
SCH = SC // 2           # 512 compressed tokens computed per core
P = 128
NCORES = 8
ET = E // P             # 8 contraction tiles for E
FT = D // P             # 8 feature tiles
CT = SC // P            # 8 compressed-token tiles
CTH = CT // 2           # 4 compressed-token tiles per half
TTOK = 512              # x streaming tile (tokens)
NG = 4                  # q groups of 512 in phase D (= conv phases)
MQ = 256                # masked q columns per group (tokens < SC)

_prog_cache = {}


def _build_program(mask_active, add_fvec, add_vbias2):
    import concourse.bacc as bacc
    import concourse.mybir as mybir
    import concourse.tile as tile

    F32 = mybir.dt.float32
    F32R = mybir.dt.float32r
    BF16 = mybir.dt.bfloat16

    nc = bacc.Bacc("TRN2", num_devices=NCORES)

    xkvT = nc.dram_tensor("xkvT", [E, HTOK], BF16, kind="ExternalInput")
    wqT = nc.dram_tensor("wqT", [E, D], BF16, kind="ExternalInput")
    WFk = nc.dram_tensor("WFk", [CF * E, D], BF16, kind="ExternalInput")
    WFv = nc.dram_tensor("WFv", [CF * E, D], BF16, kind="ExternalInput")
    bq = nc.dram_tensor("bq", [P, FT], F32, kind="ExternalInput")
    bkc = nc.dram_tensor("bkc", [P, FT], F32, kind="ExternalInput")
    maskM = None
    if mask_active:
        maskM = nc.dram_tensor("maskM", [SC, NG * MQ], BF16,
                               kind="ExternalInput")
    fvec = None
    if add_fvec:
        fvec = nc.dram_tensor("fvec", [P, D], F32, kind="ExternalInput")
    vb2 = None
    if add_vbias2:
        vb2 = nc.dram_tensor("vb2", [P, D], F32, kind="ExternalInput")
    y = nc.dram_tensor("y", [SQ, D], F32, kind="ExternalOutput")

    PAIRS = [[0, 1], [2, 3], [4, 5], [6, 7]]

    with tile.TileContext(nc) as tc, ExitStack() as top:
        persist = top.enter_context(tc.tile_pool(name="persist", bufs=1))
        dramp = top.enter_context(tc.tile_pool(name="dramp", bufs=1,
                                               space="DRAM"))
        kcT = persist.tile([P, FT, SC], F32R)       # [dh%128, dh-tile, ct]
        vcp = persist.tile([P, CT, D], F32R)        # [ct%128, ct-tile, o2]
        ones_f32 = persist.tile([P, 2], F32, tag="ones_f32")
        nc.vector.memset(ones_f32, 1.0)
        ones_sb = persist.tile([P, 2], F32R)
        nc.vector.tensor_copy(out=ones_sb, in_=ones_f32)
        bq_sb = persist.tile([P, FT], F32, tag="bq")
        bkc_sb = persist.tile([P, FT], F32, tag="bkc")
        nc.sync.dma_start(out=bq_sb, in_=bq[:])
        nc.sync.dma_start(out=bkc_sb, in_=bkc[:])
        fvec_sb = None
        if add_fvec:
            fvec_sb = persist.tile([P, D], F32, tag="fvec")
            nc.sync.dma_start(out=fvec_sb, in_=fvec[:])
        vb2_sb = None
        if add_vbias2:
            vb2_sb = persist.tile([P, D], F32, tag="vb2")
            nc.sync.dma_start(out=vb2_sb, in_=vb2[:])

        # collective bounce buffers (pair AllGather of the k/v halves)
        in_kc = dramp.tile([P, FT * SCH], F32R, tag="in_kc")
        out_kc = dramp.tile([2, P, FT * SCH], F32R, tag="out_kc")
        in_vc = dramp.tile([P, CTH * D], F32R, tag="in_vc")
        out_vc = dramp.tile([2, P, CTH * D], F32R, tag="out_vc")

        qpool = top.enter_context(tc.tile_pool(name="qpool", bufs=1))
        mkp = None
        if mask_active:
            mkp = top.enter_context(tc.tile_pool(name="mkp", bufs=1))

        with ExitStack() as kvq:
            xp = kvq.enter_context(tc.tile_pool(name="xp", bufs=1))
            x_sb = xp.tile([P, ET, HTOK], BF16, tag="x")
            # load x in (conv-phase, e-tile) order so the k GEMM can start
            # after the first 256 KB lands
            for c in range(CF):
                for et in range(ET):
                    nc.sync.dma_start(
                        out=x_sb[:, et, c * SCH:(c + 1) * SCH],
                        in_=xkvT[et * P:(et + 1) * P,
                                 c * SCH:(c + 1) * SCH])

            # ------- phases K and V: folded project+compress half --------
            def kv_phase(which):
                w_comp = WFk if which == "k" else WFv
                with ExitStack() as ph:
                    lc = ph.enter_context(
                        tc.tile_pool(name=f"l{which}", bufs=1))
                    ws = ph.enter_context(
                        tc.tile_pool(name=f"s{which}", bufs=3))
                    pp = ph.enter_context(
                        tc.tile_pool(name=f"p{which}", bufs=8, space="PSUM"))
                    if which == "k":
                        kc_loc = lc.tile([P, FT, SCH], F32R, tag="kc_loc")
                        pcs = [pp.tile([P, SCH], F32, tag="mm",
                                       name=f"pc{fo}") for fo in range(FT)]
                        for cet in range(CF * ET):
                            c, et = divmod(cet, ET)
                            w_sl = ws.tile([P, D], BF16, tag="ws")
                            nc.sync.dma_start(
                                out=w_sl,
                                in_=w_comp[cet * P:(cet + 1) * P, :])
                            rhs = x_sb[:, et, c * SCH:(c + 1) * SCH]
                            for fo in range(FT):
                                nc.tensor.matmul(
                                    pcs[fo],
                                    w_sl[:, fo * P:(fo + 1) * P],
                                    rhs,
                                    start=(cet == 0),
                                    stop=(cet == CF * ET - 1))
                        for fo in range(FT):
                            nc.vector.tensor_scalar_add(
                                out=kc_loc[:, fo, :],
                                in0=pcs[fo], scalar1=bkc_sb[:, fo:fo + 1])
                        nc.sync.dma_start(out=in_kc, in_=kc_loc)
                        if _NOCC:
                            nc.sync.dma_start(out=out_kc[0], in_=in_kc)
                            nc.sync.dma_start(out=out_kc[1], in_=in_kc)
                        else:
                            nc.gpsimd.collective_compute(
                                "AllGather", mybir.AluOpType.bypass,
                                replica_groups=PAIRS,
                                ins=[in_kc[:].opt()], outs=[out_kc[:].opt()])
                        for g in range(2):
                            nc.sync.dma_start(
                                out=kcT[:, :, g * SCH:(g + 1) * SCH],
                                in_=out_kc[g])
                    else:
                        vc_loc = lc.tile([P, CTH, D], F32R, tag="vc_loc")
                        pvs = [[pp.tile([P, SCH], F32, tag="mm",
                                        name=f"pv{ctp}_{o2s}")
                                for o2s in range(2)] for ctp in range(CTH)]
                        for cet in range(CF * ET):
                            c, et = divmod(cet, ET)
                            w_sl = ws.tile([P, D], BF16, tag="ws")
                            nc.sync.dma_start(
                                out=w_sl,
                                in_=w_comp[cet * P:(cet + 1) * P, :])
                            win = x_sb[:, et, c * SCH:(c + 1) * SCH]
                            for ctp in range(CTH):
                                lhsT = win[:, ctp * P:(ctp + 1) * P]
                                for o2s in range(2):
                                    nc.tensor.matmul(
                                        pvs[ctp][o2s],
                                        lhsT,
                                        w_sl[:, o2s * 512:(o2s + 1) * 512],
                                        start=(cet == 0),
                                        stop=(cet == CF * ET - 1))
                        for ctp in range(CTH):
                            for o2s in range(2):
                                dst = vc_loc[:, ctp,
                                             o2s * 512:(o2s + 1) * 512]
                                if add_vbias2:
                                    nc.vector.tensor_tensor(
                                        out=dst, in0=pvs[ctp][o2s],
                                        in1=vb2_sb[:,
                                                   o2s * 512:(o2s + 1) * 512],
                                        op=mybir.AluOpType.add)
                                else:
                                    nc.vector.tensor_copy(
                                        out=dst, in_=pvs[ctp][o2s])
                        nc.sync.dma_start(out=in_vc, in_=vc_loc)
                        if _NOCC:
                            nc.sync.dma_start(out=out_vc[0], in_=in_vc)
                            nc.sync.dma_start(out=out_vc[1], in_=in_vc)
                        else:
                            nc.gpsimd.collective_compute(
                                "AllGather", mybir.AluOpType.bypass,
                                replica_groups=PAIRS,
                                ins=[in_vc[:].opt()], outs=[out_vc[:].opt()])
                        for g in range(2):
                            nc.sync.dma_start(
                                out=vcp[:, g * CTH:(g + 1) * CTH, :],
                                in_=out_vc[g])

            kv_phase("k")
            kv_phase("v")

            # ------ phase Q: project q rows (scale prefolded), reusing
            # the de-interleaved bf16 x still resident in SBUF ------------
            qT = qpool.tile([P, ET, SQ], F32R, name="qT")
            mk = None
            if mask_active:
                # prefetch the (column-permuted) mask during q projection
                mk = mkp.tile([P, CT, NG * MQ], BF16)
                for ctt in range(CT):
                    nc.sync.dma_start(
                        out=mk[:, ctt, :],
                        in_=maskM[ctt * P:(ctt + 1) * P, :])
            with ExitStack() as ph:
                wp = ph.enter_context(tc.tile_pool(name="wq", bufs=1))
                pp = ph.enter_context(
                    tc.tile_pool(name="pq", bufs=8, space="PSUM"))
                w_sb = wp.tile([P, ET, D], BF16, tag="w")
                for et in range(ET):
                    nc.sync.dma_start(
                        out=w_sb[:, et, :], in_=wqT[et * P:(et + 1) * P, :])
                for tt in range(SQ // TTOK):
                    t0 = tt * TTOK
                    pts = [pp.tile([P, TTOK], F32, tag="mm", name=f"pq{fo}")
                           for fo in range(FT)]
                    for et in range(ET):
                        for fo in range(FT):
                            nc.tensor.matmul(
                                pts[fo],
                                w_sb[:, et, fo * P:(fo + 1) * P],
                                x_sb[:, et, t0:t0 + TTOK],
                                start=(et == 0), stop=(et == ET - 1))
                    for fo in range(FT):
                        nc.vector.tensor_scalar_add(
                            out=qT[:, fo, t0:t0 + TTOK],
                            in0=pts[fo], scalar1=bq_sb[:, fo:fo + 1])

        # ---------------- phase D: attention ----------------
        with ExitStack() as ph:
                att = ph.enter_context(tc.tile_pool(name="att", bufs=2))
                yp = ph.enter_context(tc.tile_pool(name="yp", bufs=3))
                rp = ph.enter_context(tc.tile_pool(name="rp", bufs=4))
                pD = ph.enter_context(
                    tc.tile_pool(name="pD", bufs=2, space="PSUM"))
                for g in range(NG):
                    q0 = g * 512
                    at = att.tile([P, CT, 512], F32R, tag="at")
                    for ctt in range(CT):
                        sc = pD.tile([P, 512], F32, tag="sc")
                        for dht in range(ET):
                            nc.tensor.matmul(
                                sc,
                                kcT[:, dht, ctt * P:(ctt + 1) * P],
                                qT[:, dht, q0:q0 + 512],
                                start=(dht == 0), stop=(dht == ET - 1))
                        nc.scalar.activation(
                            out=at[:, ctt, :], in_=sc,
                            func=mybir.ActivationFunctionType.Exp)
                        if mask_active:
                            # columns 0..255 of every group hold the
                            # tokens < SC (the tril-masked rows)
                            nc.vector.tensor_tensor(
                                out=at[:, ctt, 0:MQ], in0=at[:, ctt, 0:MQ],
                                in1=mk[:, ctt, g * MQ:(g + 1) * MQ],
                                op=mybir.AluOpType.mult)
                    for qp in range(4):
                        po = pD.tile([P, D], F32, tag="out")
                        psm = pD.tile([P, 2], F32, tag="sums")
                        for ctt in range(CT):
                            lhsT = at[:, ctt, qp * P:(qp + 1) * P]
                            nc.tensor.matmul(
                                po[:, 0:512], lhsT, vcp[:, ctt, 0:512],
                                start=(ctt == 0), stop=(ctt == CT - 1))
                            nc.tensor.matmul(
                                po[:, 512:1024], lhsT, vcp[:, ctt, 512:1024],
                                start=(ctt == 0), stop=(ctt == CT - 1))
                            nc.tensor.matmul(
                                psm, lhsT, ones_sb,
                                start=(ctt == 0), stop=(ctt == CT - 1))
                        rinv = rp.tile([P, 1], F32, tag="rinv")
                        nc.vector.reciprocal(out=rinv, in_=psm[:, 0:1])
                        y_sb = yp.tile([P, D], F32, tag="y")
                        nc.vector.tensor_scalar_mul(out=y_sb, in0=po,
                                                    scalar1=rinv)
                        if add_fvec:
                            nc.vector.tensor_tensor(
                                out=y_sb, in0=y_sb, in1=fvec_sb,
                                op=mybir.AluOpType.add)
                        # un-permute: partition i holds token 512*qp+4*i+g
                        r0 = 512 * qp + g
                        nc.sync.dma_start(out=y[r0:r0 + 509:4, :], in_=y_sb)

    nc.compile()
    return nc


def _get_program(mask_active, add_fvec, add_vbias2):
    key = (mask_active, add_fvec, add_vbias2)
    if key not in _prog_cache:
        _prog_cache[key] = _build_program(*key)
    return _prog_cache[key]


def prepare(x, w_qkv, b_qkv, wk_conv, bk_conv, wv_conv, bv_conv, w_out, b_out,
            mask):
    """Host-side prep: returns (nc, in_maps) for run_bass_kernel_spmd."""
    x = np.ascontiguousarray(np.asarray(x, np.float32))
    w_qkv = np.asarray(w_qkv, np.float32)
    b_qkv = np.asarray(b_qkv, np.float32)
    wk_conv = np.asarray(wk_conv, np.float32)
    bk_conv = np.asarray(bk_conv, np.float32)
    wv_conv = np.asarray(wv_conv, np.float32)
    bv_conv = np.asarray(bv_conv, np.float32)
    w_out = np.asarray(w_out, np.float32)
    b_out = np.asarray(b_out, np.float32)
    mask_active = bool(np.asarray(mask).reshape(-1)[0])

    scale = 1.0 / math.sqrt(D)
    wT = np.ascontiguousarray(w_qkv.T)                 # [E, 3D]
    wqT = np.ascontiguousarray((wT[:, 0:D] * scale).astype(BF))
    wkm = wT[:, D:2 * D]                               # [E, D]
    wvm = wT[:, 2 * D:3 * D]
    bkv = b_qkv[D:2 * D]
    bvv = b_qkv[2 * D:3 * D]
    bq = np.ascontiguousarray((b_qkv[0:D] * scale).reshape(FT, P).T)
    # W2[cd, o] with cd = c*D + d  <-  w_conv[o, d, c]
    W2kT = np.ascontiguousarray(wk_conv.transpose(2, 1, 0).reshape(CF * D, D))
    W2vT = np.ascontiguousarray(wv_conv.transpose(2, 1, 0).reshape(CF * D, D))
    W3 = W2vT @ w_out.T                                # fold out-proj into v
    # fold the k/v projections into the compress GEMMs:
    #   WF[(c,e), o] = sum_d wproj[e, d] * W2[(c,d), o]
    WFk = np.ascontiguousarray(
        (wkm @ W2kT.reshape(CF, D, D)).reshape(CF * E, D).astype(BF))
    WFv = np.ascontiguousarray(
        (wvm @ W3.reshape(CF, D, D)).reshape(CF * E, D).astype(BF))
    # projection biases flow through the conv contraction
    bkc2 = bk_conv + W2kT.reshape(CF, D, D).sum(0).T @ bkv
    b_vc2 = w_out @ bv_conv + W3.reshape(CF, D, D).sum(0).T @ bvv
    bkc = np.ascontiguousarray(bkc2.reshape(FT, P).T)
    add_vbias2 = bool(np.any(b_vc2))
    add_fvec = bool(np.any(b_out))

    nc = _get_program(mask_active, add_fvec, add_vbias2)

    if mask_active:
        # column-permuted tril mask: group g, col s  <->  token 4s+g
        ct_idx = np.arange(SC)[:, None]
        cols = np.concatenate(
            [4 * np.arange(MQ) + g for g in range(NG)])    # [NG*MQ]
        mm_real = np.ascontiguousarray(
            (ct_idx <= cols[None, :]).astype(BF))
        mm_ones = np.ones((SC, NG * MQ), BF)

    in_maps = []
    for core in range(NCORES):
        b, h = divmod(core, 2)
        xh = x[b].T[:, h * HTOK:(h + 1) * HTOK]
        # de-interleave the conv windows: col c*512+s holds token 4s+c
        xkv = np.ascontiguousarray(
            xh.reshape(E, HTOK // CF, CF).transpose(0, 2, 1)
            .reshape(E, HTOK).astype(BF))
        m = {
            "xkvT": xkv,
            "wqT": wqT, "WFk": WFk, "WFv": WFv,
            "bq": bq, "bkc": bkc,
        }
        if mask_active:
            m["maskM"] = mm_real if h == 0 else mm_ones
        if add_fvec:
            m["fvec"] = np.ascontiguousarray(
                np.broadcast_to(b_out[None, :], (P, D)))
        if add_vbias2:
            m["vb2"] = np.ascontiguousarray(
                np.broadcast_to(b_vc2[None, :], (P, D)))
        in_maps.append(m)
    return nc, in_maps


def assemble(results):
    out = np.empty((B, S, D), np.float32)
    for core in range(NCORES):
        b, h = divmod(core, 2)
        out[b, h * SQ:(h + 1) * SQ, :] = results[core]["y"]
    return out


def kernel(x, w_qkv, b_qkv, wk_conv, bk_conv, wv_conv, bv_conv, w_out, b_out,
           mask):
    from concourse.bass_utils import run_bass_kernel_spmd

    nc, in_maps = prepare(x, w_qkv, b_qkv, wk_conv, bk_conv, wv_conv, bv_conv,
                          w_out, b_out, mask)
    res = run_bass_kernel_spmd(nc, in_maps, core_ids=list(range(NCORES)))
    return assemble(res.results)
